# revision 3
# baseline (speedup 1.0000x reference)
"""Edge-parallel ExtractorMLP (gather + 3-layer MLP) for 8 TRN2 NeuronCores.

Strategy (pure edge parallelism, no cross-core communication):
  - 800K edges are split 100K per core.
  - The full embedding table is replicated per core as emb.T in fp16
    ([128 hidden partitions x 50000 nodes], 100KB/partition) and kept
    SBUF-resident, so both per-edge endpoint gathers are on-chip GPSIMD
    indirect_copy column gathers (hidden on partitions, edges on the free
    dim) - no transpose is needed anywhere and HBM sees each embedding
    byte exactly once.
  - The GPSIMD indirect-copy ucode only addresses data blocks up to
    ~16KB per partition, so the node axis is split into NCHUNK ranges
    (<=8192 nodes each) and each core's edges are bucketed by their
    (col_chunk, row_chunk) pair. Each 512-edge tile belongs to one
    bucket, so its two gathers read fixed table slices with chunk-local
    uint16 indices. Edge order is restored on the host afterwards.
  - The MLP runs per 512-edge tile on the tensor engine in fp16 with
    fp32 PSUM accumulation: layer 1 as 4 M-chunks x 2 K-chunks (K-chunk
    0 is the col gather, chunk 1 the row gather), layer 2 as 4 K-chunks,
    layer 3 as a single [128,1] stationary matmul. Bias+ReLU epilogues
    are split between the scalar (ACT) and vector (DVE) engines.
"""

from contextlib import ExitStack

import numpy as np

import concourse.bacc as bacc
import concourse.tile as tile
from concourse import mybir
from concourse.bass_utils import run_bass_kernel_spmd

P = 128
N = 512            # edges per tile (one fp32 PSUM bank)
IDXW = N // 16     # wrapped-index columns per tile
N_CORES = 8
N_NODES = 50000
N_EDGES = 800000
E_CORE = N_EDGES // N_CORES
NCHUNK = 7         # node-axis chunks; max chunk size must stay <= 8192

F16 = mybir.dt.float16
F32 = mybir.dt.float32
U16 = mybir.dt.uint16

_BOUNDS = np.linspace(0, N_NODES, NCHUNK + 1).astype(np.int64)


def _build_kernel(tile_map: tuple, n_nodes: int):
    """tile_map: per-tile (col_chunk, row_chunk) ids, compile-time."""
    nc = bacc.Bacc("TRN2", target_bir_lowering=False, debug=False)
    n_tiles = len(tile_map)

    tbl = nc.dram_tensor("tbl", [P, n_nodes], F16, kind="ExternalInput")
    colw = nc.dram_tensor("colw", [P, n_tiles * IDXW], U16, kind="ExternalInput")
    roww = nc.dram_tensor("roww", [P, n_tiles * IDXW], U16, kind="ExternalInput")
    w1 = nc.dram_tensor("w1", [P, 1024], F16, kind="ExternalInput")
    w2 = nc.dram_tensor("w2", [P, 512], F16, kind="ExternalInput")
    w3 = nc.dram_tensor("w3", [P, 1], F16, kind="ExternalInput")
    b1 = nc.dram_tensor("b1", [P, 4], F32, kind="ExternalInput")
    b2 = nc.dram_tensor("b2", [P, 1], F32, kind="ExternalInput")
    b3 = nc.dram_tensor("b3", [1, 1], F32, kind="ExternalInput")
    out = nc.dram_tensor("out", [n_tiles, N], F32, kind="ExternalOutput")

    Relu = mybir.ActivationFunctionType.Relu
    Identity = mybir.ActivationFunctionType.Identity

    with tile.TileContext(nc) as tc, ExitStack() as ctx:
        tblp = ctx.enter_context(tc.tile_pool(name="tblp", bufs=1))
        idxp = ctx.enter_context(tc.tile_pool(name="idxp", bufs=1))
        wp = ctx.enter_context(tc.tile_pool(name="wp", bufs=1))
        gp = ctx.enter_context(tc.tile_pool(name="gp", bufs=3))
        x1p = ctx.enter_context(tc.tile_pool(name="x1p", bufs=8))
        x2p = ctx.enter_context(tc.tile_pool(name="x2p", bufs=2))
        op = ctx.enter_context(tc.tile_pool(name="op", bufs=4))
        pl1 = ctx.enter_context(tc.tile_pool(name="pl1", bufs=4, space="PSUM"))
        pl2 = ctx.enter_context(tc.tile_pool(name="pl2", bufs=2, space="PSUM"))
        pl3 = ctx.enter_context(tc.tile_pool(name="pl3", bufs=2, space="PSUM"))

        # ---- one-time loads -------------------------------------------
        tbl_sb = tblp.tile([P, n_nodes], F16)
        n_dma = 8
        cs = (n_nodes + n_dma - 1) // n_dma
        for c in range(n_dma):
            lo, hi = c * cs, min((c + 1) * cs, n_nodes)
            if lo >= hi:
                break
            nc.sync.dma_start(tbl_sb[:, lo:hi], tbl[:, lo:hi])

        colw_sb = idxp.tile([P, n_tiles * IDXW], U16)
        roww_sb = idxp.tile([P, n_tiles * IDXW], U16)
        nc.scalar.dma_start(colw_sb[:], colw[:])
        nc.scalar.dma_start(roww_sb[:], roww[:])

        w1_sb = wp.tile([P, 1024], F16)
        w2_sb = wp.tile([P, 512], F16)
        w3_sb = wp.tile([P, 1], F16)
        b1_sb = wp.tile([P, 4], F32)
        b2_sb = wp.tile([P, 1], F32)
        b3_sb = wp.tile([1, 1], F32)
        nc.scalar.dma_start(w1_sb[:], w1[:])
        nc.scalar.dma_start(w2_sb[:], w2[:])
        nc.scalar.dma_start(w3_sb[:], w3[:])
        nc.scalar.dma_start(b1_sb[:], b1[:])
        nc.scalar.dma_start(b2_sb[:], b2[:])
        nc.scalar.dma_start(b3_sb[:], b3[:])

        bounds = [int(b) for b in _BOUNDS]

        # ---- per-tile steady state ------------------------------------
        for t, (c1, c2) in enumerate(tile_map):
            isl = slice(t * IDXW, (t + 1) * IDXW)
            g_col = gp.tile([P, N], F16, tag="gcol")
            nc.gpsimd.indirect_copy(
                g_col[:], data=tbl_sb[:, bounds[c1]:bounds[c1 + 1]],
                idxs=colw_sb[:, isl],
                i_know_ap_gather_is_preferred=True,
            )
            g_row = gp.tile([P, N], F16, tag="grow")
            nc.gpsimd.indirect_copy(
                g_row[:], data=tbl_sb[:, bounds[c2]:bounds[c2 + 1]],
                idxs=roww_sb[:, isl],
                i_know_ap_gather_is_preferred=True,
            )

            # layer 1: [E,256] @ [256,512]; K-chunk 0 = col, 1 = row
            x1s = []
            for m in range(4):
                p1 = pl1.tile([P, N], F32, tag="pl1")
                nc.tensor.matmul(
                    p1[:], lhsT=w1_sb[:, m * 128:(m + 1) * 128],
                    rhs=g_col[:], start=True, stop=False,
                )
                nc.tensor.matmul(
                    p1[:], lhsT=w1_sb[:, 512 + m * 128: 512 + (m + 1) * 128],
                    rhs=g_row[:], start=False, stop=True,
                )
                x1 = x1p.tile([P, N], F16, tag="x1")
                if m < 2:
                    nc.scalar.activation(
                        x1[:], p1[:], Relu, bias=b1_sb[:, m:m + 1]
                    )
                else:
                    nc.vector.tensor_scalar(
                        out=x1[:], in0=p1[:],
                        scalar1=b1_sb[:, m:m + 1], scalar2=0.0,
                        op0=mybir.AluOpType.add, op1=mybir.AluOpType.max,
                    )
                x1s.append(x1)

            # layer 2: [E,512] @ [512,128]
            p2 = pl2.tile([P, N], F32, tag="pl2")
            for k in range(4):
                nc.tensor.matmul(
                    p2[:], lhsT=w2_sb[:, k * 128:(k + 1) * 128],
                    rhs=x1s[k][:], start=(k == 0), stop=(k == 3),
                )
            x2 = x2p.tile([P, N], F16, tag="x2")
            nc.scalar.activation(x2[:], p2[:], Relu, bias=b2_sb[:, 0:1])

            # layer 3: [E,128] @ [128,1]
            p3 = pl3.tile([P, N], F32, tag="pl3")
            nc.tensor.matmul(p3[:1, :], lhsT=w3_sb[:], rhs=x2[:],
                             start=True, stop=True)
            o = op.tile([1, N], F32, tag="o")
            nc.scalar.activation(o[:1, :], p3[:1, :], Identity,
                                 bias=b3_sb[:1, 0:1])
            nc.sync.dma_start(out[t:t + 1, :], o[:])

    nc.compile()
    return nc


def _wrap_indices(idx: np.ndarray) -> np.ndarray:
    """[n_tiles*512] local ids -> [128, n_tiles*32] uint16 wrapped layout.

    indirect_copy unwraps each 16-partition group as
    rearrange("p s -> (s p)"), so index j of tile t sits at
    [16g + j%16, t*32 + j//16], replicated over the 8 groups g.
    """
    n_tiles = idx.shape[0] // N
    w = idx.astype(np.uint16).reshape(n_tiles, IDXW, 16).transpose(0, 2, 1)
    w = np.tile(w, (1, 8, 1))
    return np.ascontiguousarray(w.transpose(1, 0, 2).reshape(P, n_tiles * IDXW))


def _bucketize(edge_index):
    """Bucket each core's edges by (col_chunk, row_chunk).

    Returns (tile_map, per-core [col_local, row_local, slot_orig]) where
    slot_orig maps padded slot -> original edge id within the core (-1 pad).
    """
    nb = NCHUNK * NCHUNK
    cores = []
    counts = np.zeros((N_CORES, nb), np.int64)
    for c in range(N_CORES):
        sl = slice(c * E_CORE, (c + 1) * E_CORE)
        col = np.asarray(edge_index[0, sl], dtype=np.int64)
        row = np.asarray(edge_index[1, sl], dtype=np.int64)
        c1 = np.searchsorted(_BOUNDS[1:-1], col, side="right")
        c2 = np.searchsorted(_BOUNDS[1:-1], row, side="right")
        key = c1 * NCHUNK + c2
        order = np.argsort(key, kind="stable")
        counts[c] = np.bincount(key, minlength=nb)
        cores.append((col, row, key, order))

    tiles_per_bucket = np.ceil(counts.max(axis=0) / N).astype(np.int64)
    tile_map = []
    bucket_tile_start = np.zeros(nb, np.int64)
    for k in range(nb):
        bucket_tile_start[k] = len(tile_map)
        tile_map.extend([(k // NCHUNK, k % NCHUNK)] * int(tiles_per_bucket[k]))
    n_tiles = len(tile_map)

    per_core = []
    for c in range(N_CORES):
        col, row, key, order = cores[c]
        col_l = np.zeros(n_tiles * N, np.int64)
        row_l = np.zeros(n_tiles * N, np.int64)
        slot_orig = np.full(n_tiles * N, -1, np.int64)
        pos = 0
        for k in range(nb):
            nk = int(counts[c, k])
            if nk == 0:
                continue
            eids = order[pos:pos + nk]
            pos += nk
            base = int(bucket_tile_start[k]) * N
            c1, c2 = k // NCHUNK, k % NCHUNK
            col_l[base:base + nk] = col[eids] - _BOUNDS[c1]
            row_l[base:base + nk] = row[eids] - _BOUNDS[c2]
            slot_orig[base:base + nk] = eids
        per_core.append((col_l, row_l, slot_orig))
    return tuple(tile_map), per_core


def _prep_shared(emb, W1, b1, W2, b2, W3, b3):
    return {
        "tbl": np.ascontiguousarray(emb.astype(np.float16).T),
        "w1": np.ascontiguousarray(
            np.concatenate([W1[:128, :], W1[128:, :]], axis=1)
        ).astype(np.float16),
        "w2": np.ascontiguousarray(
            np.concatenate([W2[k * 128:(k + 1) * 128, :] for k in range(4)],
                           axis=1)
        ).astype(np.float16),
        "w3": W3.astype(np.float16),
        "b1": np.ascontiguousarray(b1.reshape(4, 128).T).astype(np.float32),
        "b2": b2[:, None].astype(np.float32),
        "b3": b3[None, :].astype(np.float32),
    }


_NC_CACHE = {}


def _get_nc(tile_map):
    key = (tile_map, N_NODES)
    if key not in _NC_CACHE:
        _NC_CACHE[key] = _build_kernel(tile_map, N_NODES)
    return _NC_CACHE[key]


def run(inputs: dict, trace: bool = False):
    """Run the kernel on 8 cores; returns (out [800000,1] f32, results)."""
    emb = np.asarray(inputs["emb"], dtype=np.float32)
    edge_index = np.asarray(inputs["edge_index"])
    shared = _prep_shared(
        emb,
        *[np.asarray(inputs[k], dtype=np.float32)
          for k in ("W1", "b1", "W2", "b2", "W3", "b3")]
    )
    tile_map, per_core = _bucketize(edge_index)
    in_maps = [
        dict(shared, colw=_wrap_indices(col_l), roww=_wrap_indices(row_l))
        for (col_l, row_l, _) in per_core
    ]
    nc = _get_nc(tile_map)
    res = run_bass_kernel_spmd(nc, in_maps, list(range(N_CORES)), trace=trace)
    out = np.empty((N_EDGES,), np.float32)
    for c in range(N_CORES):
        flat = res.results[c]["out"].reshape(-1)
        slot_orig = per_core[c][2]
        valid = slot_orig >= 0
        core_out = np.empty((E_CORE,), np.float32)
        core_out[slot_orig[valid]] = flat[valid]
        out[c * E_CORE:(c + 1) * E_CORE] = core_out
    return out[:, None], res


def kernel(**inputs) -> np.ndarray:
    out, _ = run(inputs, trace=False)
    return out


# revision 4
# speedup vs baseline: 1.0108x; 1.0108x over previous
"""Edge-parallel ExtractorMLP (gather + 3-layer MLP) for 8 TRN2 NeuronCores.

Strategy (pure edge parallelism, no cross-core communication):
  - 800K edges are split 100K per core.
  - The full embedding table is replicated per core as emb.T in fp16
    ([128 hidden partitions x 50000 nodes], 100KB/partition) and kept
    SBUF-resident, so both per-edge endpoint gathers are on-chip GPSIMD
    indirect_copy column gathers (hidden on partitions, edges on the free
    dim) - no transpose is needed anywhere and HBM sees each embedding
    byte exactly once.
  - The GPSIMD indirect-copy ucode only addresses data blocks up to
    ~16KB per partition, so the node axis is split into NCHUNK ranges
    (<=8192 nodes each) and each core's edges are bucketed by their
    (col_chunk, row_chunk) pair. Each 512-edge tile belongs to one
    bucket, so its two gathers read fixed table slices with chunk-local
    uint16 indices. Edge order is restored on the host afterwards.
  - The MLP runs per 512-edge tile on the tensor engine in fp16 with
    fp32 PSUM accumulation: layer 1 as 4 M-chunks x 2 K-chunks (K-chunk
    0 is the col gather, chunk 1 the row gather), layer 2 as 4 K-chunks,
    layer 3 as a single [128,1] stationary matmul. Bias+ReLU epilogues
    are split between the scalar (ACT) and vector (DVE) engines.
"""

from contextlib import ExitStack

import numpy as np

import concourse.bacc as bacc
import concourse.tile as tile
from concourse import mybir
from concourse.bass_utils import run_bass_kernel_spmd

P = 128
N = 512            # edges per tile (one fp32 PSUM bank)
IDXW = N // 16     # wrapped-index columns per tile
N_CORES = 8
N_NODES = 50000
N_EDGES = 800000
E_CORE = N_EDGES // N_CORES
NCHUNK = 7         # node-axis chunks; max chunk size must stay <= 8192
GGRP = 2           # tiles per gather (indirect_copy caps at 1024 indices)

F16 = mybir.dt.float16
F32 = mybir.dt.float32
U16 = mybir.dt.uint16

_BOUNDS = np.linspace(0, N_NODES, NCHUNK + 1).astype(np.int64)


def _build_kernel(tile_map: tuple, n_nodes: int):
    """tile_map: per-tile (col_chunk, row_chunk) ids, compile-time."""
    nc = bacc.Bacc("TRN2", target_bir_lowering=False, debug=False)
    n_tiles = len(tile_map)

    tbl = nc.dram_tensor("tbl", [P, n_nodes], F16, kind="ExternalInput")
    colw = nc.dram_tensor("colw", [P, n_tiles * IDXW], U16, kind="ExternalInput")
    roww = nc.dram_tensor("roww", [P, n_tiles * IDXW], U16, kind="ExternalInput")
    w1 = nc.dram_tensor("w1", [P, 1024], F16, kind="ExternalInput")
    w2 = nc.dram_tensor("w2", [P, 512], F16, kind="ExternalInput")
    w3 = nc.dram_tensor("w3", [P, 1], F16, kind="ExternalInput")
    b1 = nc.dram_tensor("b1", [P, 4], F32, kind="ExternalInput")
    b2 = nc.dram_tensor("b2", [P, 1], F32, kind="ExternalInput")
    b3 = nc.dram_tensor("b3", [1, 1], F32, kind="ExternalInput")
    out = nc.dram_tensor("out", [n_tiles, N], F32, kind="ExternalOutput")

    Relu = mybir.ActivationFunctionType.Relu
    Identity = mybir.ActivationFunctionType.Identity

    with tile.TileContext(nc) as tc, ExitStack() as ctx:
        tblp = ctx.enter_context(tc.tile_pool(name="tblp", bufs=1))
        idxp = ctx.enter_context(tc.tile_pool(name="idxp", bufs=1))
        wp = ctx.enter_context(tc.tile_pool(name="wp", bufs=1))
        gp = ctx.enter_context(tc.tile_pool(name="gp", bufs=3))
        x1p = ctx.enter_context(tc.tile_pool(name="x1p", bufs=8))
        x2p = ctx.enter_context(tc.tile_pool(name="x2p", bufs=2))
        op = ctx.enter_context(tc.tile_pool(name="op", bufs=4))
        pl1 = ctx.enter_context(tc.tile_pool(name="pl1", bufs=5, space="PSUM"))
        pl2 = ctx.enter_context(tc.tile_pool(name="pl2", bufs=2, space="PSUM"))
        pl3 = ctx.enter_context(tc.tile_pool(name="pl3", bufs=1, space="PSUM"))

        # ---- one-time loads -------------------------------------------
        tbl_sb = tblp.tile([P, n_nodes], F16)
        n_dma = 8
        cs = (n_nodes + n_dma - 1) // n_dma
        for c in range(n_dma):
            lo, hi = c * cs, min((c + 1) * cs, n_nodes)
            if lo >= hi:
                break
            nc.sync.dma_start(tbl_sb[:, lo:hi], tbl[:, lo:hi])

        colw_sb = idxp.tile([P, n_tiles * IDXW], U16)
        roww_sb = idxp.tile([P, n_tiles * IDXW], U16)
        nc.scalar.dma_start(colw_sb[:], colw[:])
        nc.scalar.dma_start(roww_sb[:], roww[:])

        w1_sb = wp.tile([P, 1024], F16)
        w2_sb = wp.tile([P, 512], F16)
        w3_sb = wp.tile([P, 1], F16)
        b1_sb = wp.tile([P, 4], F32)
        b2_sb = wp.tile([P, 1], F32)
        b3_sb = wp.tile([1, 1], F32)
        nc.scalar.dma_start(w1_sb[:], w1[:])
        nc.scalar.dma_start(w2_sb[:], w2[:])
        nc.scalar.dma_start(w3_sb[:], w3[:])
        nc.scalar.dma_start(b1_sb[:], b1[:])
        nc.scalar.dma_start(b2_sb[:], b2[:])
        nc.scalar.dma_start(b3_sb[:], b3[:])

        bounds = [int(b) for b in _BOUNDS]

        # gather groups: runs of up to GGRP tiles within one bucket
        groups = []
        t = 0
        while t < n_tiles:
            g = 1
            while (g < GGRP and t + g < n_tiles
                   and tile_map[t + g] == tile_map[t]):
                g += 1
            groups.append((t, g))
            t += g

        # ---- steady state ---------------------------------------------
        for t0, gsz in groups:
            c1, c2 = tile_map[t0]
            isl = slice(t0 * IDXW, (t0 + gsz) * IDXW)
            g_col = gp.tile([P, GGRP * N], F16, tag="gcol")
            nc.gpsimd.indirect_copy(
                g_col[:, :gsz * N], data=tbl_sb[:, bounds[c1]:bounds[c1 + 1]],
                idxs=colw_sb[:, isl],
                i_know_ap_gather_is_preferred=True,
            )
            g_row = gp.tile([P, GGRP * N], F16, tag="grow")
            nc.gpsimd.indirect_copy(
                g_row[:, :gsz * N], data=tbl_sb[:, bounds[c2]:bounds[c2 + 1]],
                idxs=roww_sb[:, isl],
                i_know_ap_gather_is_preferred=True,
            )

            for j in range(gsz):
                t = t0 + j
                act_first = (t % 2 == 0)
                jsl = slice(j * N, (j + 1) * N)

                # layer 1: [E,256] @ [256,512]; K-chunk 0 = col, 1 = row
                x1s = []
                for m in range(4):
                    p1 = pl1.tile([P, N], F32, tag="pl1")
                    nc.tensor.matmul(
                        p1[:], lhsT=w1_sb[:, m * 128:(m + 1) * 128],
                        rhs=g_col[:, jsl], start=True, stop=False,
                    )
                    nc.tensor.matmul(
                        p1[:], lhsT=w1_sb[:, 512 + m * 128: 512 + (m + 1) * 128],
                        rhs=g_row[:, jsl], start=False, stop=True,
                    )
                    x1 = x1p.tile([P, N], F16, tag="x1")
                    if (m < 2) == act_first:
                        nc.scalar.activation(
                            x1[:], p1[:], Relu, bias=b1_sb[:, m:m + 1]
                        )
                    else:
                        nc.vector.tensor_scalar(
                            out=x1[:], in0=p1[:],
                            scalar1=b1_sb[:, m:m + 1], scalar2=0.0,
                            op0=mybir.AluOpType.add, op1=mybir.AluOpType.max,
                        )
                    x1s.append(x1)

                # layer 2: [E,512] @ [512,128]
                p2 = pl2.tile([P, N], F32, tag="pl2")
                for k in range(4):
                    nc.tensor.matmul(
                        p2[:], lhsT=w2_sb[:, k * 128:(k + 1) * 128],
                        rhs=x1s[k][:], start=(k == 0), stop=(k == 3),
                    )
                x2 = x2p.tile([P, N], F16, tag="x2")
                if act_first:
                    nc.scalar.activation(x2[:], p2[:], Relu, bias=b2_sb[:, 0:1])
                else:
                    nc.vector.tensor_scalar(
                        out=x2[:], in0=p2[:],
                        scalar1=b2_sb[:, 0:1], scalar2=0.0,
                        op0=mybir.AluOpType.add, op1=mybir.AluOpType.max,
                    )

                # layer 3: [E,128] @ [128,1]
                p3 = pl3.tile([P, N], F32, tag="pl3")
                nc.tensor.matmul(p3[:1, :], lhsT=w3_sb[:], rhs=x2[:],
                                 start=True, stop=True)
                o = op.tile([1, N], F32, tag="o")
                if act_first:
                    nc.vector.tensor_scalar(
                        out=o[:1, :], in0=p3[:1, :], scalar1=b3_sb[:1, 0:1],
                        scalar2=None, op0=mybir.AluOpType.add,
                    )
                else:
                    nc.scalar.activation(o[:1, :], p3[:1, :], Identity,
                                         bias=b3_sb[:1, 0:1])
                nc.sync.dma_start(out[t:t + 1, :], o[:])

    nc.compile()
    return nc


def _wrap_indices(idx: np.ndarray) -> np.ndarray:
    """[n_tiles*512] local ids -> [128, n_tiles*32] uint16 wrapped layout.

    indirect_copy unwraps each 16-partition group as
    rearrange("p s -> (s p)"), so index j of tile t sits at
    [16g + j%16, t*32 + j//16], replicated over the 8 groups g.
    """
    n_tiles = idx.shape[0] // N
    w = idx.astype(np.uint16).reshape(n_tiles, IDXW, 16).transpose(0, 2, 1)
    w = np.tile(w, (1, 8, 1))
    return np.ascontiguousarray(w.transpose(1, 0, 2).reshape(P, n_tiles * IDXW))


def _bucketize(edge_index):
    """Bucket each core's edges by (col_chunk, row_chunk).

    Returns (tile_map, per-core [col_local, row_local, slot_orig]) where
    slot_orig maps padded slot -> original edge id within the core (-1 pad).
    """
    nb = NCHUNK * NCHUNK
    cores = []
    counts = np.zeros((N_CORES, nb), np.int64)
    for c in range(N_CORES):
        sl = slice(c * E_CORE, (c + 1) * E_CORE)
        col = np.asarray(edge_index[0, sl], dtype=np.int64)
        row = np.asarray(edge_index[1, sl], dtype=np.int64)
        c1 = np.searchsorted(_BOUNDS[1:-1], col, side="right")
        c2 = np.searchsorted(_BOUNDS[1:-1], row, side="right")
        key = c1 * NCHUNK + c2
        order = np.argsort(key, kind="stable")
        counts[c] = np.bincount(key, minlength=nb)
        cores.append((col, row, key, order))

    tiles_per_bucket = np.ceil(counts.max(axis=0) / N).astype(np.int64)
    tile_map = []
    bucket_tile_start = np.zeros(nb, np.int64)
    for k in range(nb):
        bucket_tile_start[k] = len(tile_map)
        tile_map.extend([(k // NCHUNK, k % NCHUNK)] * int(tiles_per_bucket[k]))
    n_tiles = len(tile_map)

    per_core = []
    for c in range(N_CORES):
        col, row, key, order = cores[c]
        col_l = np.zeros(n_tiles * N, np.int64)
        row_l = np.zeros(n_tiles * N, np.int64)
        slot_orig = np.full(n_tiles * N, -1, np.int64)
        pos = 0
        for k in range(nb):
            nk = int(counts[c, k])
            if nk == 0:
                continue
            eids = order[pos:pos + nk]
            pos += nk
            base = int(bucket_tile_start[k]) * N
            c1, c2 = k // NCHUNK, k % NCHUNK
            col_l[base:base + nk] = col[eids] - _BOUNDS[c1]
            row_l[base:base + nk] = row[eids] - _BOUNDS[c2]
            slot_orig[base:base + nk] = eids
        per_core.append((col_l, row_l, slot_orig))
    return tuple(tile_map), per_core


def _prep_shared(emb, W1, b1, W2, b2, W3, b3):
    return {
        "tbl": np.ascontiguousarray(emb.astype(np.float16).T),
        "w1": np.ascontiguousarray(
            np.concatenate([W1[:128, :], W1[128:, :]], axis=1)
        ).astype(np.float16),
        "w2": np.ascontiguousarray(
            np.concatenate([W2[k * 128:(k + 1) * 128, :] for k in range(4)],
                           axis=1)
        ).astype(np.float16),
        "w3": W3.astype(np.float16),
        "b1": np.ascontiguousarray(b1.reshape(4, 128).T).astype(np.float32),
        "b2": b2[:, None].astype(np.float32),
        "b3": b3[None, :].astype(np.float32),
    }


_NC_CACHE = {}


def _get_nc(tile_map):
    key = (tile_map, N_NODES)
    if key not in _NC_CACHE:
        _NC_CACHE[key] = _build_kernel(tile_map, N_NODES)
    return _NC_CACHE[key]


def run(inputs: dict, trace: bool = False):
    """Run the kernel on 8 cores; returns (out [800000,1] f32, results)."""
    emb = np.asarray(inputs["emb"], dtype=np.float32)
    edge_index = np.asarray(inputs["edge_index"])
    shared = _prep_shared(
        emb,
        *[np.asarray(inputs[k], dtype=np.float32)
          for k in ("W1", "b1", "W2", "b2", "W3", "b3")]
    )
    tile_map, per_core = _bucketize(edge_index)
    in_maps = [
        dict(shared, colw=_wrap_indices(col_l), roww=_wrap_indices(row_l))
        for (col_l, row_l, _) in per_core
    ]
    nc = _get_nc(tile_map)
    res = run_bass_kernel_spmd(nc, in_maps, list(range(N_CORES)), trace=trace)
    out = np.empty((N_EDGES,), np.float32)
    for c in range(N_CORES):
        flat = res.results[c]["out"].reshape(-1)
        slot_orig = per_core[c][2]
        valid = slot_orig >= 0
        core_out = np.empty((E_CORE,), np.float32)
        core_out[slot_orig[valid]] = flat[valid]
        out[c * E_CORE:(c + 1) * E_CORE] = core_out
    return out[:, None], res


def kernel(**inputs) -> np.ndarray:
    out, _ = run(inputs, trace=False)
    return out


# revision 5
# speedup vs baseline: 1.0109x; 1.0001x over previous
"""Edge-parallel ExtractorMLP (gather + 3-layer MLP) for 8 TRN2 NeuronCores.

Strategy (pure edge parallelism, no cross-core communication):
  - 800K edges are split 100K per core.
  - The full embedding table is replicated per core as emb.T in fp16
    ([128 hidden partitions x 50000 nodes], 100KB/partition) and kept
    SBUF-resident, so both per-edge endpoint gathers are on-chip GPSIMD
    indirect_copy column gathers (hidden on partitions, edges on the free
    dim) - no transpose is needed anywhere and HBM sees each embedding
    byte exactly once.
  - The GPSIMD indirect-copy ucode only addresses data blocks up to
    ~16KB per partition, so the node axis is split into NCHUNK ranges
    (<=8192 nodes each) and each core's edges are bucketed by their
    (col_chunk, row_chunk) pair. Each 512-edge tile belongs to one
    bucket, so its two gathers read fixed table slices with chunk-local
    uint16 indices. Edge order is restored on the host afterwards.
  - The MLP runs per 512-edge tile on the tensor engine in fp16 with
    fp32 PSUM accumulation: layer 1 as 4 M-chunks x 2 K-chunks (K-chunk
    0 is the col gather, chunk 1 the row gather), layer 2 as 4 K-chunks,
    layer 3 as a single [128,1] stationary matmul. Bias+ReLU epilogues
    are split between the scalar (ACT) and vector (DVE) engines.
"""

from contextlib import ExitStack

import numpy as np

import concourse.bacc as bacc
import concourse.tile as tile
from concourse import mybir
from concourse.bass_utils import run_bass_kernel_spmd

P = 128
N = 512            # edges per tile (one fp32 PSUM bank)
IDXW = N // 16     # wrapped-index columns per tile
N_CORES = 8
N_NODES = 50000
N_EDGES = 800000
E_CORE = N_EDGES // N_CORES
NCHUNK = 7         # node-axis chunks; max chunk size must stay <= 8192
GGRP = 2           # tiles per gather (indirect_copy caps at 1024 indices)

F16 = mybir.dt.float16
F32 = mybir.dt.float32
U16 = mybir.dt.uint16

_BOUNDS = np.linspace(0, N_NODES, NCHUNK + 1).astype(np.int64)


def _build_kernel(tile_map: tuple, n_nodes: int):
    """tile_map: per-tile (col_chunk, row_chunk) ids, compile-time."""
    nc = bacc.Bacc("TRN2", target_bir_lowering=False, debug=False)
    n_tiles = len(tile_map)

    tbl = nc.dram_tensor("tbl", [P, n_nodes], F16, kind="ExternalInput")
    colw = nc.dram_tensor("colw", [P, n_tiles * IDXW], U16, kind="ExternalInput")
    roww = nc.dram_tensor("roww", [P, n_tiles * IDXW], U16, kind="ExternalInput")
    w1 = nc.dram_tensor("w1", [P, 1024], F16, kind="ExternalInput")
    w2 = nc.dram_tensor("w2", [P, 512], F16, kind="ExternalInput")
    w3 = nc.dram_tensor("w3", [P, 1], F16, kind="ExternalInput")
    b1 = nc.dram_tensor("b1", [P, 4], F32, kind="ExternalInput")
    b2 = nc.dram_tensor("b2", [P, 1], F32, kind="ExternalInput")
    b3 = nc.dram_tensor("b3", [1, 1], F32, kind="ExternalInput")
    out = nc.dram_tensor("out", [n_tiles, N], F32, kind="ExternalOutput")

    Relu = mybir.ActivationFunctionType.Relu
    Identity = mybir.ActivationFunctionType.Identity

    with tile.TileContext(nc) as tc, ExitStack() as ctx:
        tblp = ctx.enter_context(tc.tile_pool(name="tblp", bufs=1))
        idxp = ctx.enter_context(tc.tile_pool(name="idxp", bufs=1))
        wp = ctx.enter_context(tc.tile_pool(name="wp", bufs=1))
        gcp = ctx.enter_context(tc.tile_pool(name="gcp", bufs=4))
        grp = ctx.enter_context(tc.tile_pool(name="grp", bufs=4))
        x1p = ctx.enter_context(tc.tile_pool(name="x1p", bufs=12))
        x2p = ctx.enter_context(tc.tile_pool(name="x2p", bufs=4))
        op = ctx.enter_context(tc.tile_pool(name="op", bufs=8))
        pl1 = ctx.enter_context(tc.tile_pool(name="pl1", bufs=5, space="PSUM"))
        pl2 = ctx.enter_context(tc.tile_pool(name="pl2", bufs=2, space="PSUM"))
        pl3 = ctx.enter_context(tc.tile_pool(name="pl3", bufs=1, space="PSUM"))

        # ---- one-time loads -------------------------------------------
        tbl_sb = tblp.tile([P, n_nodes], F16)
        n_dma = 8
        cs = (n_nodes + n_dma - 1) // n_dma
        for c in range(n_dma):
            lo, hi = c * cs, min((c + 1) * cs, n_nodes)
            if lo >= hi:
                break
            nc.sync.dma_start(tbl_sb[:, lo:hi], tbl[:, lo:hi])

        colw_sb = idxp.tile([P, n_tiles * IDXW], U16)
        roww_sb = idxp.tile([P, n_tiles * IDXW], U16)
        nc.scalar.dma_start(colw_sb[:], colw[:])
        nc.scalar.dma_start(roww_sb[:], roww[:])

        w1_sb = wp.tile([P, 1024], F16)
        w2_sb = wp.tile([P, 512], F16)
        w3_sb = wp.tile([P, 1], F16)
        b1_sb = wp.tile([P, 4], F32)
        b2_sb = wp.tile([P, 1], F32)
        b3_sb = wp.tile([1, 1], F32)
        nc.scalar.dma_start(w1_sb[:], w1[:])
        nc.scalar.dma_start(w2_sb[:], w2[:])
        nc.scalar.dma_start(w3_sb[:], w3[:])
        nc.scalar.dma_start(b1_sb[:], b1[:])
        nc.scalar.dma_start(b2_sb[:], b2[:])
        nc.scalar.dma_start(b3_sb[:], b3[:])

        bounds = [int(b) for b in _BOUNDS]

        # gather groups: runs of up to GGRP tiles within one bucket
        groups = []
        t = 0
        while t < n_tiles:
            g = 1
            while (g < GGRP and t + g < n_tiles
                   and tile_map[t + g] == tile_map[t]):
                g += 1
            groups.append((t, g))
            t += g

        # ---- steady state ---------------------------------------------
        for t0, gsz in groups:
            c1, c2 = tile_map[t0]
            isl = slice(t0 * IDXW, (t0 + gsz) * IDXW)
            g_col = gcp.tile([P, GGRP * N], F16, tag="gcol")
            nc.gpsimd.indirect_copy(
                g_col[:, :gsz * N], data=tbl_sb[:, bounds[c1]:bounds[c1 + 1]],
                idxs=colw_sb[:, isl],
                i_know_ap_gather_is_preferred=True,
            )
            g_row = grp.tile([P, GGRP * N], F16, tag="grow")
            nc.gpsimd.indirect_copy(
                g_row[:, :gsz * N], data=tbl_sb[:, bounds[c2]:bounds[c2 + 1]],
                idxs=roww_sb[:, isl],
                i_know_ap_gather_is_preferred=True,
            )

            for j in range(gsz):
                t = t0 + j
                act_first = (t % 2 == 0)
                jsl = slice(j * N, (j + 1) * N)

                # layer 1: [E,256] @ [256,512]; K-chunk 0 = col, 1 = row
                x1s = []
                for m in range(4):
                    p1 = pl1.tile([P, N], F32, tag="pl1")
                    nc.tensor.matmul(
                        p1[:], lhsT=w1_sb[:, m * 128:(m + 1) * 128],
                        rhs=g_col[:, jsl], start=True, stop=False,
                    )
                    nc.tensor.matmul(
                        p1[:], lhsT=w1_sb[:, 512 + m * 128: 512 + (m + 1) * 128],
                        rhs=g_row[:, jsl], start=False, stop=True,
                    )
                    x1 = x1p.tile([P, N], F16, tag="x1")
                    if (m < 2) == act_first:
                        nc.scalar.activation(
                            x1[:], p1[:], Relu, bias=b1_sb[:, m:m + 1]
                        )
                    else:
                        nc.vector.tensor_scalar(
                            out=x1[:], in0=p1[:],
                            scalar1=b1_sb[:, m:m + 1], scalar2=0.0,
                            op0=mybir.AluOpType.add, op1=mybir.AluOpType.max,
                        )
                    x1s.append(x1)

                # layer 2: [E,512] @ [512,128]
                p2 = pl2.tile([P, N], F32, tag="pl2")
                for k in range(4):
                    nc.tensor.matmul(
                        p2[:], lhsT=w2_sb[:, k * 128:(k + 1) * 128],
                        rhs=x1s[k][:], start=(k == 0), stop=(k == 3),
                    )
                x2 = x2p.tile([P, N], F16, tag="x2")
                if act_first:
                    nc.scalar.activation(x2[:], p2[:], Relu, bias=b2_sb[:, 0:1])
                else:
                    nc.vector.tensor_scalar(
                        out=x2[:], in0=p2[:],
                        scalar1=b2_sb[:, 0:1], scalar2=0.0,
                        op0=mybir.AluOpType.add, op1=mybir.AluOpType.max,
                    )

                # layer 3: [E,128] @ [128,1]
                p3 = pl3.tile([P, N], F32, tag="pl3")
                nc.tensor.matmul(p3[:1, :], lhsT=w3_sb[:], rhs=x2[:],
                                 start=True, stop=True)
                o = op.tile([1, N], F32, tag="o")
                if act_first:
                    nc.vector.tensor_scalar(
                        out=o[:1, :], in0=p3[:1, :], scalar1=b3_sb[:1, 0:1],
                        scalar2=None, op0=mybir.AluOpType.add,
                    )
                else:
                    nc.scalar.activation(o[:1, :], p3[:1, :], Identity,
                                         bias=b3_sb[:1, 0:1])
                nc.sync.dma_start(out[t:t + 1, :], o[:])

    nc.compile()
    return nc


def _wrap_indices(idx: np.ndarray) -> np.ndarray:
    """[n_tiles*512] local ids -> [128, n_tiles*32] uint16 wrapped layout.

    indirect_copy unwraps each 16-partition group as
    rearrange("p s -> (s p)"), so index j of tile t sits at
    [16g + j%16, t*32 + j//16], replicated over the 8 groups g.
    """
    n_tiles = idx.shape[0] // N
    w = idx.astype(np.uint16).reshape(n_tiles, IDXW, 16).transpose(0, 2, 1)
    w = np.tile(w, (1, 8, 1))
    return np.ascontiguousarray(w.transpose(1, 0, 2).reshape(P, n_tiles * IDXW))


def _bucketize(edge_index):
    """Bucket each core's edges by (col_chunk, row_chunk).

    Returns (tile_map, per-core [col_local, row_local, slot_orig]) where
    slot_orig maps padded slot -> original edge id within the core (-1 pad).
    """
    nb = NCHUNK * NCHUNK
    cores = []
    counts = np.zeros((N_CORES, nb), np.int64)
    for c in range(N_CORES):
        sl = slice(c * E_CORE, (c + 1) * E_CORE)
        col = np.asarray(edge_index[0, sl], dtype=np.int64)
        row = np.asarray(edge_index[1, sl], dtype=np.int64)
        c1 = np.searchsorted(_BOUNDS[1:-1], col, side="right")
        c2 = np.searchsorted(_BOUNDS[1:-1], row, side="right")
        key = c1 * NCHUNK + c2
        order = np.argsort(key, kind="stable")
        counts[c] = np.bincount(key, minlength=nb)
        cores.append((col, row, key, order))

    tiles_per_bucket = np.ceil(counts.max(axis=0) / N).astype(np.int64)
    tile_map = []
    bucket_tile_start = np.zeros(nb, np.int64)
    for k in range(nb):
        bucket_tile_start[k] = len(tile_map)
        tile_map.extend([(k // NCHUNK, k % NCHUNK)] * int(tiles_per_bucket[k]))
    n_tiles = len(tile_map)

    per_core = []
    for c in range(N_CORES):
        col, row, key, order = cores[c]
        col_l = np.zeros(n_tiles * N, np.int64)
        row_l = np.zeros(n_tiles * N, np.int64)
        slot_orig = np.full(n_tiles * N, -1, np.int64)
        pos = 0
        for k in range(nb):
            nk = int(counts[c, k])
            if nk == 0:
                continue
            eids = order[pos:pos + nk]
            pos += nk
            base = int(bucket_tile_start[k]) * N
            c1, c2 = k // NCHUNK, k % NCHUNK
            col_l[base:base + nk] = col[eids] - _BOUNDS[c1]
            row_l[base:base + nk] = row[eids] - _BOUNDS[c2]
            slot_orig[base:base + nk] = eids
        per_core.append((col_l, row_l, slot_orig))
    return tuple(tile_map), per_core


def _prep_shared(emb, W1, b1, W2, b2, W3, b3):
    return {
        "tbl": np.ascontiguousarray(emb.astype(np.float16).T),
        "w1": np.ascontiguousarray(
            np.concatenate([W1[:128, :], W1[128:, :]], axis=1)
        ).astype(np.float16),
        "w2": np.ascontiguousarray(
            np.concatenate([W2[k * 128:(k + 1) * 128, :] for k in range(4)],
                           axis=1)
        ).astype(np.float16),
        "w3": W3.astype(np.float16),
        "b1": np.ascontiguousarray(b1.reshape(4, 128).T).astype(np.float32),
        "b2": b2[:, None].astype(np.float32),
        "b3": b3[None, :].astype(np.float32),
    }


_NC_CACHE = {}


def _get_nc(tile_map):
    key = (tile_map, N_NODES)
    if key not in _NC_CACHE:
        _NC_CACHE[key] = _build_kernel(tile_map, N_NODES)
    return _NC_CACHE[key]


def run(inputs: dict, trace: bool = False):
    """Run the kernel on 8 cores; returns (out [800000,1] f32, results)."""
    emb = np.asarray(inputs["emb"], dtype=np.float32)
    edge_index = np.asarray(inputs["edge_index"])
    shared = _prep_shared(
        emb,
        *[np.asarray(inputs[k], dtype=np.float32)
          for k in ("W1", "b1", "W2", "b2", "W3", "b3")]
    )
    tile_map, per_core = _bucketize(edge_index)
    in_maps = [
        dict(shared, colw=_wrap_indices(col_l), roww=_wrap_indices(row_l))
        for (col_l, row_l, _) in per_core
    ]
    nc = _get_nc(tile_map)
    res = run_bass_kernel_spmd(nc, in_maps, list(range(N_CORES)), trace=trace)
    out = np.empty((N_EDGES,), np.float32)
    for c in range(N_CORES):
        flat = res.results[c]["out"].reshape(-1)
        slot_orig = per_core[c][2]
        valid = slot_orig >= 0
        core_out = np.empty((E_CORE,), np.float32)
        core_out[slot_orig[valid]] = flat[valid]
        out[c * E_CORE:(c + 1) * E_CORE] = core_out
    return out[:, None], res


def kernel(**inputs) -> np.ndarray:
    out, _ = run(inputs, trace=False)
    return out
